# revision 29
# baseline (speedup 1.0000x reference)
"""Trainium2 Bass kernel: causal self-attention (B=2, T=2048, C=1024, H=16, Dh=64).

Sharding: 8 cores = 2 (batch) x 4 (head groups of 4 heads).  Each core gets
x[b] plus the W_qkv rows / W_proj columns for its heads, computes the full
attention + a partial output projection for its batch, and the host sums the
4 partials per batch (tensor-parallel unshard).

All matmuls run in bf16 with f32 PSUM accumulation.  x is passed transposed
(xT = x[b].T) so that:
  qT, kT = Wq @ xT, Wk @ xT     (head dim on partitions)  -- no transposes
  v      = xT.T @ WvT           (natural [T, d] layout)
  S^T    = kT_h(tile).T @ qT_h  ([k, q] layout, 128x512 blocks, the two
           heads of a pair row-tiled into array halves)
  exp on ScalarE (logits are bounded, no max pass needed); causal masking by
  computing only the live columns of each block plus one multiplicative
  [128,128] triangle mask on the diagonal subtile (gpsimd); row sums via a
  ones column appended to V (P@[V|1] accumulates y^T and the softmax
  denominators in one PSUM tile).
  out_partial = y^T.T @ WpT   (f32, DMA'd out).

Schedule: the PE executes its queue IN ORDER, so QKV tiles are emitted
through a deadline-ordered work queue sprinkled one-per-iteration into the
attention i-loops (plus previous-chunk projection tiles), never as a bulk
phase.  This keeps the PE continuously busy and the HAM activity monitor at
K=8/8 (full 2.4GHz clock).  Y matmuls are emitted one iteration behind their
exp (software pipelining) so the in-order PE queue never parks on an
unfinished ACTIVATE.

Input DMAs are split across four issuing engines (sync/scalar/vector/
gpsimd) so the ~6MB prefix lands in parallel hardware queues; the QKV weight
images are oc-major so the first 128-output-column slice of Wq/Wk is one
contiguous transfer and the first matmul can start ~4us in.

Softmax normalization (all chunks): denominator row 64 of the y PSUM tile
goes through reciprocal_approx_fast on the DVE (~5x the iterative-divide
RECIPROCAL; 18 good bits is plenty for a softmax denominator), a gpsimd
partition_broadcast over 64 partitions, and a gpsimd multiply into yT.  No
DRAM round-trip: the old den->DRAM->8-lane-reciprocal->DRAM->broadcast path
parked the DVE queue on a ~4us DMA chain, which blocked the y-PSUM
evictions the next chunk's first Y matmul needed.  The final chunk runs the
same chain with the multiply on the DVE and warm-filler matmuls bridging
the ~3us chain latency (keeps HAM at full clock for the last projection
burst); the burst's output DMAs split at 256-column granularity across
three hardware queues.
"""
import sys
import types

import numpy as np
import ml_dtypes

_BF16 = ml_dtypes.bfloat16


def _install_ntff_hook():
    """Provide antenv.axon_hooks so run_bass_kernel_spmd(trace=True) works."""
    if "antenv.axon_hooks" in sys.modules:
        return
    mod = types.ModuleType("antenv.axon_hooks")
    mod._hook = None

    def set_axon_ntff_profile_hook(h):
        mod._hook = h

    def get_axon_ntff_profile_hook():
        return mod._hook

    mod.set_axon_ntff_profile_hook = set_axon_ntff_profile_hook
    mod.get_axon_ntff_profile_hook = get_axon_ntff_profile_hook
    sys.modules["antenv.axon_hooks"] = mod
    try:
        import antenv

        antenv.axon_hooks = mod
    except Exception:
        pass
    try:
        from trn_agent_boot.trn_boot import _ntff_profile_via_ctypes

        mod.set_axon_ntff_profile_hook(
            _ntff_profile_via_ctypes("/opt/axon/libaxon_pjrt.so")
        )
    except Exception:
        pass


_install_ntff_hook()

import concourse.bacc as bacc
import concourse.mybir as mybir
from concourse import bass_utils
from concourse.tile import TileContext

# no network bucket in this container; keep artifacts local
bass_utils.upload_artifacts = lambda tmpdir: tmpdir

BF16 = mybir.dt.bfloat16
F32 = mybir.dt.float32

B, T, C = 2, 2048, 1024
H, D = 16, 64
HL = 4            # heads per core
OL = HL * D       # 256 local qkv output dim
P = 128
KC = C // P       # 8 contraction chunks
NQT = T // P      # 16 q/k 128-tiles
NQC = T // 512    # 4 q 512-chunks
VA = D + 1        # v columns per head incl. ones column (65)
OCW = KC * P      # 1024: per-oc column block in the oc-major weight image

_nc_cache = None


def _build_nc():
    nc = bacc.Bacc("TRN2", target_bir_lowering=False, debug=False, num_devices=8)

    # all inputs arrive pre-arranged in SBUF-image layout [128, X] so every
    # input DMA moves multi-KB contiguous runs per partition row.
    # wq/wk are oc-major ([p, oc, kc, 128]) so the oc=0 half is contiguous.
    xT = nc.declare_dram_parameter("xT", [P, KC * T], BF16, isOutput=False)
    wqT = nc.declare_dram_parameter("wqT", [P, 2 * OCW], BF16, isOutput=False)
    wkT = nc.declare_dram_parameter("wkT", [P, 2 * OCW], BF16, isOutput=False)
    wvT = nc.declare_dram_parameter("wvT", [P, KC * OL], BF16, isOutput=False)
    wpT = nc.declare_dram_parameter("wpT", [P, 2 * C], BF16, isOutput=False)
    mk = nc.declare_dram_parameter("mask_tri", [P, P], BF16, isOutput=False)
    # head-pair select matrix for the broadcast matmul: row 0 = 1 on cols
    # 0:64, row 32 = 1 on cols 64:128, all else 0 (engine APs must start at
    # partition multiples of 32, so the two reciprocal rows live at
    # partitions 0 and 32 of a [64,512] tile)
    bsel = nc.declare_dram_parameter("bsel", [D, P], BF16, isOutput=False)
    # bf16 partials (summed in f32 on the host): halves eviction + output
    # DMA cost; adds ~0.4% relative rounding, well inside the 2e-2 budget
    out = nc.declare_dram_parameter("out", [T, C], BF16, isOutput=True)

    Exp = mybir.ActivationFunctionType.Exp

    with TileContext(nc) as tc:
        with tc.tile_pool(name="const", bufs=1) as const, \
             tc.tile_pool(name="misc", bufs=3) as misc, \
             tc.tile_pool(name="att", bufs=8) as att, \
             tc.tile_pool(name="outp", bufs=6) as outp:
            xT_sb = const.tile([P, KC * T], BF16, name="xT_sb")
            wq_sb = const.tile([P, 2 * OCW], BF16, name="wq_sb")
            wk_sb = const.tile([P, 2 * OCW], BF16, name="wk_sb")
            wv_sb = const.tile([P, KC * OL], BF16, name="wv_sb")
            wp_sb = const.tile([P, 2 * C], BF16, name="wp_sb")
            mk_sb = const.tile([P, P], BF16, name="mk_sb")
            qT_sb = const.tile([P, 2 * T], BF16, name="qT_sb")
            kT_sb = const.tile([P, 2 * T], BF16, name="kT_sb")
            va_sb = const.tile([P, NQT * HL * VA], BF16, name="va_sb")
            yT_sb = const.tile([P, 2 * T], BF16, name="yT_sb")
            bs_sb = const.tile([D, P], BF16, name="bs_sb")
            # contiguous staging for the final projection burst: 4 row
            # blocks x 1024 cols land as 4 clean [128,1024] DMAs
            stg_sb = const.tile([P, 4 * C], BF16, name="stg_sb")
            # persistent reciprocal tiles: rows 0 and 32 carry the two
            # heads' denominators; all other rows stay 1.0 so the shared
            # in-place reciprocal + cast never see garbage
            rc2_sb = const.tile([D, 512], F32, name="rc2_sb")
            rcb2_sb = const.tile([D, 512], BF16, name="rcb2_sb")
            wsc_sb = misc.tile([P, 512], BF16, name="wsc_sb", tag="wsc")

            # ---- input DMAs: issued across THREE engines (sync/scalar/
            # gpsimd; the DVE cannot issue DMAs) so the prefix lands in
            # parallel hardware queues.  First-needed tiles first: mask
            # (warmups), wq oc=0, x tch=0 (split 4 ways), wk oc=0.
            xv = xT_sb[:, :].rearrange("p (n t) -> p n t", n=KC)
            xs = xT[:, :].rearrange("p (n t) -> p n t", n=KC)
            nc.vector.memset(wsc_sb[:, :], 0.0)
            nc.sync.dma_start(out=mk_sb[:, :], in_=mk[:, :])
            nc.scalar.dma_start(out=wq_sb[:, 0:OCW], in_=wqT[:, 0:OCW])
            nc.gpsimd.dma_start(out=xv[:, 6:8, 0:512], in_=xs[:, 6:8, 0:512])
            nc.sync.dma_start(out=xv[:, 0:3, 0:512], in_=xs[:, 0:3, 0:512])
            nc.scalar.dma_start(out=xv[:, 3:6, 0:512], in_=xs[:, 3:6, 0:512])
            nc.gpsimd.dma_start(out=wv_sb[:, :], in_=wvT[:, :])
            nc.sync.dma_start(out=bs_sb[:, :], in_=bsel[:, :])
            nc.sync.dma_start(out=wk_sb[:, 0:OCW], in_=wkT[:, 0:OCW])
            nc.scalar.dma_start(out=wk_sb[:, OCW:], in_=wkT[:, OCW:])
            nc.gpsimd.dma_start(out=wq_sb[:, OCW:], in_=wqT[:, OCW:])
            nc.sync.dma_start(out=xv[:, 0:4, 512:1024], in_=xs[:, 0:4, 512:1024])
            nc.gpsimd.dma_start(out=xv[:, 4:8, 512:1024], in_=xs[:, 4:8, 512:1024])
            nc.sync.dma_start(out=xv[:, 0:4, 1024:T], in_=xs[:, 0:4, 1024:T])
            nc.gpsimd.dma_start(out=xv[:, 4:8, 1024:T], in_=xs[:, 4:8, 1024:T])
            nc.scalar.dma_start(out=wp_sb[:, :], in_=wpT[:, :])
            va_view = va_sb[:, :].rearrange("p (t h e) -> p t h e", t=NQT, h=HL)
            nc.vector.memset(va_view[:, :, :, D:VA], 1.0)
            nc.vector.memset(rc2_sb[:, :], 1.0)

            # ---- merged QKV + attention + projection pipeline ----
            # The PE executes its queue IN ORDER, so emitting all of QKV
            # before attention serializes them.  Instead: a minimal QKV
            # prefix, then the remaining QKV tiles flow through a
            # deadline-ordered work queue sprinkled into the attention
            # i-loops.  PSUM rings (8 banks): s 2x2, y 2x1, aux 2x1.
            with tc.tile_pool(name="s_ps", bufs=2, space="PSUM") as s_pool, \
                 tc.tile_pool(name="y_ps", bufs=2, space="PSUM") as y_pool, \
                 tc.tile_pool(name="aux_ps", bufs=2, space="PSUM") as aux_pool:
                # PE warm-up: the HAM clock gate only reaches 8/8 after
                # ~3.4us of sustained activity and the first real matmul
                # can't start until the input DMA lands; burn the window on
                # throwaway matmuls over the first-loaded mask.
                wps = aux_pool.tile([P, 512], F32, name="warmps", tag="aux")
                for w in range(10):
                    nc.tensor.matmul(
                        wps[:, :], mk_sb[:, :], wsc_sb[:, :],
                        start=True, stop=True,
                    )

                def warm_fill(n):
                    # throwaway matmuls that keep the HAM duty-cycle high
                    # across a known PE bubble (dependency-free, so they
                    # execute exactly when the queue would otherwise stall)
                    w_ps = aux_pool.tile([P, 512], F32, name="wfps", tag="aux")
                    for _ in range(n):
                        nc.tensor.matmul(
                            w_ps[:, :], mk_sb[:, :], wsc_sb[:, :],
                            start=True, stop=True,
                        )

                def qk_tile(w_sb, dst_sb, oc, tch):
                    ps = aux_pool.tile([P, 512], F32, name="qkps", tag="aux")
                    for kc in range(KC):
                        nc.tensor.matmul(
                            ps[:, :],
                            w_sb[:, oc * OCW + kc * P: oc * OCW + kc * P + P],
                            xT_sb[:, kc * T + tch * 512: kc * T + tch * 512 + 512],
                            start=(kc == 0),
                            stop=(kc == KC - 1),
                        )
                    # DVE eviction: ScalarE runs nothing but exp
                    nc.vector.tensor_copy(
                        dst_sb[:, oc * T + tch * 512: oc * T + tch * 512 + 512],
                        ps[:, :],
                    )

                def v_tile(tt):
                    ps = aux_pool.tile([P, 512], F32, name="vps", tag="aux")
                    for kc in range(KC):
                        nc.tensor.matmul(
                            ps[:, 0:OL],
                            xT_sb[:, kc * T + tt * P: kc * T + tt * P + P],
                            wv_sb[:, kc * OL:(kc + 1) * OL],
                            start=(kc == 0),
                            stop=(kc == KC - 1),
                        )
                    nc.vector.tensor_copy(
                        va_view[:, tt, :, 0:D],
                        ps[:, 0:OL].rearrange("p (h d) -> p h d", h=HL),
                    )

                fine_eng = [nc.sync, nc.scalar]
                fine_ctr = [0]

                def proj_tile(tile_idx, fine=False):
                    tt, ocn = divmod(tile_idx, 2)
                    trow = tt * P
                    if fine and fine_ctr[0] % 4 >= 2:
                        # final burst: alternate PSUM pools (aux + the
                        # now-free y ring) so 4 accumulations are in flight
                        # and the PE never waits on an eviction
                        pr_ps = y_pool.tile([P, 512], F32, name="prps2", tag="yps")
                    else:
                        pr_ps = aux_pool.tile([P, 512], F32, name="prps", tag="aux")
                    for cc in range(2):
                        nc.tensor.matmul(
                            pr_ps[:, :],
                            yT_sb[:, cc * T + trow: cc * T + trow + P],
                            wp_sb[:, cc * C + ocn * 512: cc * C + ocn * 512 + 512],
                            start=(cc == 0),
                            stop=(cc == 1),
                        )
                    if not fine:
                        o_sb = outp.tile([P, 512], BF16, name="osb", tag="osb")
                        nc.vector.tensor_copy(o_sb[:, :], pr_ps[:, :])
                        # alternate DMA-issuing engine: descriptors land in
                        # two hardware queues, so the output tiles drain in
                        # parallel instead of serializing
                        if tile_idx % 2 == 0:
                            nc.sync.dma_start(
                                out=out[trow:trow + P, ocn * 512:(ocn + 1) * 512],
                                in_=o_sb[:, :],
                            )
                        else:
                            nc.scalar.dma_start(
                                out=out[trow:trow + P, ocn * 512:(ocn + 1) * 512],
                                in_=o_sb[:, :],
                            )
                    else:
                        # final burst: evict into the contiguous staging
                        # tile (alternating DVE / ScalarE so eviction keeps
                        # pace with the PE); each tile's DMA is issued on
                        # sync right after its eviction so transfers fire
                        # as soon as their semaphore bumps
                        k = fine_ctr[0]
                        fine_ctr[0] += 1
                        dst = stg_sb[:, k * 512:(k + 1) * 512]
                        if k % 2 == 0:
                            nc.vector.tensor_copy(dst, pr_ps[:, :])
                        else:
                            nc.scalar.copy(dst, pr_ps[:, :])
                        nc.sync.dma_start(
                            out=out[trow:trow + P, ocn * 512:(ocn + 1) * 512],
                            in_=dst,
                        )

                # minimal prefix: exactly what attention chunk (0,0)'s first
                # S matmul needs (v tiles flow through the work queue -- the
                # first Y matmul only runs ~2 exps later)
                qk_tile(wq_sb, qT_sb, 0, 0)
                qk_tile(wk_sb, kT_sb, 0, 0)

                # the rest of QKV, deadline-ordered by the first chunk that
                # consumes each tile; popped one per attention iteration
                def mk_qk(w_sb, dst_sb, oc, tch):
                    return lambda: qk_tile(w_sb, dst_sb, oc, tch)

                def mk_v(tt):
                    return lambda: v_tile(tt)

                work = [mk_v(0), mk_v(1), mk_v(2), mk_v(3),
                        mk_qk(wq_sb, qT_sb, 1, 0), mk_qk(wk_sb, kT_sb, 1, 0)]
                for tch in (1, 2, 3):
                    work += [mk_qk(wq_sb, qT_sb, 0, tch),
                             mk_qk(wk_sb, kT_sb, 0, tch)]
                    work += [mk_v(tt) for tt in range(4 * tch, 4 * tch + 4)]
                    work += [mk_qk(wq_sb, qT_sb, 1, tch),
                             mk_qk(wk_sb, kT_sb, 1, tch)]
                # units that must be emitted before chunk (j4, hp) starts
                req = {(0, 0): 0, (0, 1): 6, (1, 0): 12, (1, 1): 14,
                       (2, 0): 20, (2, 1): 22, (3, 0): 28, (3, 1): 30}
                seq = [(a, b) for a in range(NQC) for b in range(2)]
                emitted = [0]
                # PE broadcast matmuls + DVE normalize-mults deferred from
                # the previous chunk's tail (emitted at i==1 of the next
                # chunk so the in-order PE queue never parks on the
                # reciprocal chain)
                carry = []

                def pop_work():
                    if work:
                        work.pop(0)()
                        emitted[0] += 1

                for j4 in range(NQC):
                    q0 = j4 * 512
                    for hp in range(2):
                        # flush any not-yet-emitted prerequisites
                        while emitted[0] < req[(j4, hp)]:
                            pop_work()
                        nxt = seq.index((j4, hp)) + 1
                        req_next = req[seq[nxt]] if nxt < len(seq) else 30
                        # previous chunk's projection tiles are sprinkled
                        # into the i-loop below: each proj MM is independent
                        # PE work that fills the S->exp->Y handoff bubble.
                        # hp=0's pend tiles need the chunk that JUST ended
                        # (its gpsimd normalize-mult lands ~1.5us in), so
                        # they start at i>=3; hp=1's pend chunk is a full
                        # pass old and can start at i>=1.
                        pend = (
                            [(j4 - 1) * 8 + hp * 4 + k for k in range(4)]
                            if j4 > 0 else []
                        )
                        pend_start = 3 if hp == 0 else 1
                        # two heads interleaved per k-tile: one shared 2-bank
                        # S tile, one wide exp for both heads (the +352cyc
                        # ACTIVATE pipeline fill amortizes over 1024 cols),
                        # two independent y accumulations.  Doubles the
                        # PE-side work available per ACT op.
                        h0, h1 = 2 * hp, 2 * hp + 1
                        ch = hp
                        y0 = y_pool.tile([P, 512], F32, name="yps0", tag="yps")
                        y1 = y_pool.tile([P, 512], F32, name="yps1", tag="yps")
                        nk = 4 * (j4 + 1)

                        def emit_y(c0, p2, i):
                            for half, y_ps, hh in ((0, y0, h0), (1, y1, h1)):
                                nc.tensor.matmul(
                                    y_ps[0:VA, c0:512],
                                    va_sb[:, (i * HL + hh) * VA:(i * HL + hh) * VA + VA],
                                    p2[:, half * 512 + c0: half * 512 + 512],
                                    start=(i == 0),
                                    stop=(i == nk - 1),
                                )

                        prev_y = None
                        for i in range(nk):
                            m0 = max(0, i - 4 * j4)
                            c0 = P * m0
                            s2 = s_pool.tile([P, 1024], F32, name="sps", tag="sps")
                            for half, po in ((0, 0), (1, 64)):
                                nc.tensor.matmul(
                                    s2[:, half * 512 + c0: half * 512 + 512],
                                    kT_sb[po:po + D, ch * T + i * P: ch * T + i * P + P],
                                    qT_sb[po:po + D, ch * T + q0 + c0: ch * T + q0 + 512],
                                    start=True,
                                    stop=True,
                                )
                            p2 = att.tile([P, 1024], BF16, name="pt", tag="pt")
                            if m0 == 0:
                                nc.scalar.activation(
                                    p2[:, 0:1024], s2[:, 0:1024], Exp, scale=0.125
                                )
                            else:
                                # diagonal: the two live spans are disjoint;
                                # one 3D-AP exp covers both (halves the
                                # +352cyc ACTIVATE fills on the diagonal)
                                s2v = s2[:, :].rearrange("p (h c) -> p h c", h=2)
                                p2v = p2[:, :].rearrange("p (h c) -> p h c", h=2)
                                nc.scalar.activation(
                                    p2v[:, :, c0:512], s2v[:, :, c0:512],
                                    Exp, scale=0.125,
                                )
                            if i >= 4 * j4:
                                for half in range(2):
                                    nc.gpsimd.tensor_mul(
                                        p2[:, half * 512 + c0: half * 512 + c0 + P],
                                        p2[:, half * 512 + c0: half * 512 + c0 + P],
                                        mk_sb[:, :],
                                    )
                            # independent PE work between S(i) and Y(i-1):
                            # a QKV tile (paced so each chunk's inputs are
                            # ready one chunk ahead) or a proj tile.  These
                            # MMs execute while exp(i-1)/exp(i) run, so the
                            # in-order PE queue never parks on a Y waiting
                            # for its exp.
                            if carry and i == 1:
                                while carry:
                                    carry.pop(0)()
                            elif emitted[0] < req_next:
                                pop_work()
                            elif pend and i % 2 == 1 and i >= pend_start:
                                proj_tile(pend.pop(0))
                            elif work and i % 2 == 0:
                                pop_work()
                            if prev_y is not None:
                                emit_y(*prev_y)
                            prev_y = (c0, p2, i)
                        emit_y(*prev_y)
                        for t in pend:
                            proj_tile(t)

                        # tail: evict both heads' y rows 0:64 (frees PSUM),
                        # pull the denominator row from PSUM to partition 0,
                        # fast approximate reciprocal (the custom DVE op
                        # needs matching in/out partitions), cast to bf16.
                        # The broadcast is a PE matmul (ones[1,64] stationary
                        # from the mask's first row x rcb[1,512] moving ->
                        # [64,512] PSUM) and the normalize-mult runs on the
                        # DVE reading that PSUM tile directly.  gpsimd runs
                        # ONLY tensor-tensor multiplies (the causal masks):
                        # mixing in PartitionBroadcast or DMA issues forces
                        # a ~7us DSP library reload per switch.
                        last = (j4 == NQC - 1 and hp == 1)
                        # combined two-head normalization: both heads' y
                        # rows evicted into ONE [128,512] tile (h0 on rows
                        # 0:64, h1 on 64:128, matching the yT layout), both
                        # denominator rows into a [2,512] tile -> one 2-lane
                        # reciprocal + bf16 cast, one select-stationary
                        # broadcast matmul (bsel.T @ rcb2 -> [128,512] PSUM)
                        # and one full-width DVE multiply into yT.
                        ypair = misc.tile([P, 512], F32, name="ysb", tag="ysb")
                        if not last:
                            # evictions first (frees the y PSUM ring for the
                            # next chunk ASAP), then the reciprocal chain;
                            # broadcast+mult deferred into the next chunk
                            nc.vector.tensor_copy(ypair[0:D, :], y0[0:D, :])
                            nc.vector.tensor_copy(ypair[D:2 * D, :], y1[0:D, :])
                        else:
                            # final chunk: y evictions on the now-idle
                            # ScalarE, in parallel with the DVE chain
                            nc.scalar.copy(ypair[0:D, :], y0[0:D, :])
                            nc.scalar.copy(ypair[D:2 * D, :], y1[0:D, :])
                        nc.vector.tensor_copy(rc2_sb[0:1, :], y0[D:VA, :])
                        nc.vector.tensor_copy(rc2_sb[32:33, :], y1[D:VA, :])
                        nc.vector.reciprocal_approx_fast(rc2_sb[:, :], rc2_sb[:, :])
                        nc.vector.tensor_copy(rcb2_sb[:, :], rc2_sb[:, :])

                        def mk_norm(ypair=ypair, ch=ch, q0=q0):
                            def emit():
                                bc_ps = aux_pool.tile(
                                    [P, 512], F32, name="bcps", tag="aux"
                                )
                                nc.tensor.matmul(
                                    bc_ps[:, :], bs_sb[:, :], rcb2_sb[:, :],
                                    start=True, stop=True,
                                )
                                nc.vector.tensor_mul(
                                    yT_sb[:, ch * T + q0: ch * T + q0 + 512],
                                    ypair[:, :],
                                    bc_ps[:, :],
                                )
                            return emit

                        if not last:
                            carry.append(mk_norm())
                        else:
                            # the final chunk's normalization gates the last
                            # projection burst and the kernel end: bridge
                            # the ~2us chain with warm fillers (keeps HAM at
                            # full clock), then the staged burst
                            warm_fill(4)
                            mk_norm()()
                            warm_fill(3)
                            for k in range(8):
                                proj_tile(j4 * 8 + k, fine=True)
    nc.compile()
    return nc


def _get_nc():
    global _nc_cache
    if _nc_cache is None:
        _nc_cache = _build_nc()
    return _nc_cache


def _prepare_in_maps(x, W_qkv, W_proj):
    x = np.asarray(x, np.float32)
    W_qkv = np.asarray(W_qkv, np.float32)
    W_proj = np.asarray(W_proj, np.float32)
    # [r, j] = 1 where j >= r (upper triangle incl diag, in S^T [k, q] layout)
    tri = (np.arange(P)[None, :] >= np.arange(P)[:, None]).astype(np.float32)
    tri = tri.astype(_BF16)
    bsel = np.zeros((D, P), np.float32)
    bsel[0, 0:D] = 1.0
    bsel[32, D:2 * D] = 1.0
    bsel = bsel.astype(_BF16)
    in_maps = []
    for c in range(8):
        b, g = c // 4, c % 4
        r0 = OL * g

        def img(a):
            # [R, Y] with R = n*128 rows -> SBUF image [128, n*Y]
            n = a.shape[0] // P
            return np.ascontiguousarray(
                a.reshape(n, P, a.shape[1]).transpose(1, 0, 2).reshape(P, -1)
            ).astype(_BF16)

        def img_oc(a):
            # [1024, 256] -> oc-major SBUF image [128, oc(2) * kc(8) * 128]
            return np.ascontiguousarray(
                a.reshape(KC, P, 2, P).transpose(1, 2, 0, 3).reshape(P, -1)
            ).astype(_BF16)

        in_maps.append({
            "xT": img(x[b].T),
            "wqT": img_oc(W_qkv[r0:r0 + OL, :].T),
            "wkT": img_oc(W_qkv[C + r0:C + r0 + OL, :].T),
            "wvT": img(W_qkv[2 * C + r0:2 * C + r0 + OL, :].T),
            "wpT": img(W_proj[:, r0:r0 + OL].T),
            "mask_tri": tri,
            "bsel": bsel,
        })
    return in_maps


def _combine(results):
    out = np.zeros((B, T, C), np.float32)
    for c in range(8):
        out[c // 4] += results[c]["out"]
    return out


def kernel(x, W_qkv, W_proj):
    nc = _get_nc()
    in_maps = _prepare_in_maps(x, W_qkv, W_proj)
    try:
        res = bass_utils.run_bass_kernel_spmd(nc, in_maps, core_ids=list(range(8)))
    except Exception:
        # rare transient NRT device errors; one retry
        res = bass_utils.run_bass_kernel_spmd(nc, in_maps, core_ids=list(range(8)))
    return _combine(res.results)


def kernel_traced(x, W_qkv, W_proj, trace_cores=None):
    """Like kernel() but returns (out, exec_time_ns) using an NTFF profile."""
    nc = _get_nc()
    in_maps = _prepare_in_maps(x, W_qkv, W_proj)
    res = bass_utils.run_bass_kernel_spmd(
        nc, in_maps, core_ids=list(range(8)), trace=True, trace_cores=trace_cores
    )
    return _combine(res.results), res.exec_time_ns


# revision 30
# speedup vs baseline: 1.0082x; 1.0082x over previous
"""Trainium2 Bass kernel: causal self-attention (B=2, T=2048, C=1024, H=16, Dh=64).

Sharding: 8 cores = 2 (batch) x 4 (head groups of 4 heads).  Each core gets
x[b] plus the W_qkv rows / W_proj columns for its heads, computes the full
attention + a partial output projection for its batch, and the host sums the
4 partials per batch (tensor-parallel unshard).

All matmuls run in bf16 with f32 PSUM accumulation.  x is passed transposed
(xT = x[b].T) so that:
  qT, kT = Wq @ xT, Wk @ xT     (head dim on partitions)  -- no transposes
  v      = xT.T @ WvT           (natural [T, d] layout)
  S^T    = kT_h(tile).T @ qT_h  ([k, q] layout, 128x512 blocks, the two
           heads of a pair row-tiled into array halves)
  exp on ScalarE (logits are bounded, no max pass needed); causal masking by
  computing only the live columns of each block plus one multiplicative
  [128,128] triangle mask on the diagonal subtile (gpsimd); row sums via a
  ones column appended to V (P@[V|1] accumulates y^T and the softmax
  denominators in one PSUM tile).
  out_partial = y^T.T @ WpT   (f32, DMA'd out).

Schedule: the PE executes its queue IN ORDER, so QKV tiles are emitted
through a deadline-ordered work queue sprinkled one-per-iteration into the
attention i-loops (plus previous-chunk projection tiles), never as a bulk
phase.  This keeps the PE continuously busy and the HAM activity monitor at
K=8/8 (full 2.4GHz clock).  Y matmuls are emitted one iteration behind their
exp (software pipelining) so the in-order PE queue never parks on an
unfinished ACTIVATE.

Input DMAs are split across four issuing engines (sync/scalar/vector/
gpsimd) so the ~6MB prefix lands in parallel hardware queues; the QKV weight
images are oc-major so the first 128-output-column slice of Wq/Wk is one
contiguous transfer and the first matmul can start ~4us in.

Softmax normalization (all chunks): denominator row 64 of the y PSUM tile
goes through reciprocal_approx_fast on the DVE (~5x the iterative-divide
RECIPROCAL; 18 good bits is plenty for a softmax denominator), a gpsimd
partition_broadcast over 64 partitions, and a gpsimd multiply into yT.  No
DRAM round-trip: the old den->DRAM->8-lane-reciprocal->DRAM->broadcast path
parked the DVE queue on a ~4us DMA chain, which blocked the y-PSUM
evictions the next chunk's first Y matmul needed.  The final chunk runs the
same chain with the multiply on the DVE and warm-filler matmuls bridging
the ~3us chain latency (keeps HAM at full clock for the last projection
burst); the burst's output DMAs split at 256-column granularity across
three hardware queues.
"""
import sys
import types

import numpy as np
import ml_dtypes

_BF16 = ml_dtypes.bfloat16


def _install_ntff_hook():
    """Provide antenv.axon_hooks so run_bass_kernel_spmd(trace=True) works."""
    if "antenv.axon_hooks" in sys.modules:
        return
    mod = types.ModuleType("antenv.axon_hooks")
    mod._hook = None

    def set_axon_ntff_profile_hook(h):
        mod._hook = h

    def get_axon_ntff_profile_hook():
        return mod._hook

    mod.set_axon_ntff_profile_hook = set_axon_ntff_profile_hook
    mod.get_axon_ntff_profile_hook = get_axon_ntff_profile_hook
    sys.modules["antenv.axon_hooks"] = mod
    try:
        import antenv

        antenv.axon_hooks = mod
    except Exception:
        pass
    try:
        from trn_agent_boot.trn_boot import _ntff_profile_via_ctypes

        mod.set_axon_ntff_profile_hook(
            _ntff_profile_via_ctypes("/opt/axon/libaxon_pjrt.so")
        )
    except Exception:
        pass


_install_ntff_hook()

import concourse.bacc as bacc
import concourse.mybir as mybir
from concourse import bass_utils
from concourse.tile import TileContext

# no network bucket in this container; keep artifacts local
bass_utils.upload_artifacts = lambda tmpdir: tmpdir

BF16 = mybir.dt.bfloat16
F32 = mybir.dt.float32

B, T, C = 2, 2048, 1024
H, D = 16, 64
HL = 4            # heads per core
OL = HL * D       # 256 local qkv output dim
P = 128
KC = C // P       # 8 contraction chunks
NQT = T // P      # 16 q/k 128-tiles
NQC = T // 512    # 4 q 512-chunks
VA = D + 1        # v columns per head incl. ones column (65)
OCW = KC * P      # 1024: per-oc column block in the oc-major weight image

_nc_cache = None


def _build_nc():
    nc = bacc.Bacc("TRN2", target_bir_lowering=False, debug=False, num_devices=8)

    # all inputs arrive pre-arranged in SBUF-image layout [128, X] so every
    # input DMA moves multi-KB contiguous runs per partition row.
    # wq/wk are oc-major ([p, oc, kc, 128]) so the oc=0 half is contiguous.
    xT = nc.declare_dram_parameter("xT", [P, KC * T], BF16, isOutput=False)
    wqT = nc.declare_dram_parameter("wqT", [P, 2 * OCW], BF16, isOutput=False)
    wkT = nc.declare_dram_parameter("wkT", [P, 2 * OCW], BF16, isOutput=False)
    wvT = nc.declare_dram_parameter("wvT", [P, KC * OL], BF16, isOutput=False)
    wpT = nc.declare_dram_parameter("wpT", [P, 2 * C], BF16, isOutput=False)
    mk = nc.declare_dram_parameter("mask_tri", [P, P], BF16, isOutput=False)
    # head-pair select matrix for the broadcast matmul: row 0 = 1 on cols
    # 0:64, row 32 = 1 on cols 64:128, all else 0 (engine APs must start at
    # partition multiples of 32, so the two reciprocal rows live at
    # partitions 0 and 32 of a [64,512] tile)
    bsel = nc.declare_dram_parameter("bsel", [D, P], BF16, isOutput=False)
    # bf16 partials (summed in f32 on the host): halves eviction + output
    # DMA cost; adds ~0.4% relative rounding, well inside the 2e-2 budget
    out = nc.declare_dram_parameter("out", [T, C], BF16, isOutput=True)

    Exp = mybir.ActivationFunctionType.Exp

    with TileContext(nc) as tc:
        with tc.tile_pool(name="const", bufs=1) as const, \
             tc.tile_pool(name="misc", bufs=3) as misc, \
             tc.tile_pool(name="att", bufs=8) as att, \
             tc.tile_pool(name="outp", bufs=6) as outp:
            xT_sb = const.tile([P, KC * T], BF16, name="xT_sb")
            wq_sb = const.tile([P, 2 * OCW], BF16, name="wq_sb")
            wk_sb = const.tile([P, 2 * OCW], BF16, name="wk_sb")
            wv_sb = const.tile([P, KC * OL], BF16, name="wv_sb")
            wp_sb = const.tile([P, 2 * C], BF16, name="wp_sb")
            mk_sb = const.tile([P, P], BF16, name="mk_sb")
            qT_sb = const.tile([P, 2 * T], BF16, name="qT_sb")
            kT_sb = const.tile([P, 2 * T], BF16, name="kT_sb")
            va_sb = const.tile([P, NQT * HL * VA], BF16, name="va_sb")
            yT_sb = const.tile([P, 2 * T], BF16, name="yT_sb")
            bs_sb = const.tile([D, P], BF16, name="bs_sb")
            # contiguous staging for the final projection burst: 4 row
            # blocks x 1024 cols land as 4 clean [128,1024] DMAs
            stg_sb = const.tile([P, 4 * C], BF16, name="stg_sb")
            # persistent reciprocal tiles: rows 0 and 32 carry the two
            # heads' denominators; all other rows stay 1.0 so the shared
            # in-place reciprocal + cast never see garbage
            rc2_sb = const.tile([D, 512], F32, name="rc2_sb")
            rcb2_sb = const.tile([D, 512], BF16, name="rcb2_sb")
            wsc_sb = misc.tile([P, 512], BF16, name="wsc_sb", tag="wsc")

            # ---- input DMAs: issued across THREE engines (sync/scalar/
            # gpsimd; the DVE cannot issue DMAs) so the prefix lands in
            # parallel hardware queues.  First-needed tiles first: mask
            # (warmups), wq oc=0, x tch=0 (split 4 ways), wk oc=0.
            xv = xT_sb[:, :].rearrange("p (n t) -> p n t", n=KC)
            xs = xT[:, :].rearrange("p (n t) -> p n t", n=KC)
            nc.vector.memset(wsc_sb[:, :], 0.0)
            nc.sync.dma_start(out=mk_sb[:, :], in_=mk[:, :])
            nc.scalar.dma_start(out=wq_sb[:, 0:OCW], in_=wqT[:, 0:OCW])
            nc.gpsimd.dma_start(out=xv[:, 6:8, 0:512], in_=xs[:, 6:8, 0:512])
            nc.sync.dma_start(out=xv[:, 0:3, 0:512], in_=xs[:, 0:3, 0:512])
            nc.scalar.dma_start(out=xv[:, 3:6, 0:512], in_=xs[:, 3:6, 0:512])
            nc.gpsimd.dma_start(out=wv_sb[:, :], in_=wvT[:, :])
            nc.sync.dma_start(out=bs_sb[:, :], in_=bsel[:, :])
            nc.sync.dma_start(out=wk_sb[:, 0:OCW], in_=wkT[:, 0:OCW])
            nc.scalar.dma_start(out=wk_sb[:, OCW:], in_=wkT[:, OCW:])
            nc.gpsimd.dma_start(out=wq_sb[:, OCW:], in_=wqT[:, OCW:])
            nc.sync.dma_start(out=xv[:, 0:4, 512:1024], in_=xs[:, 0:4, 512:1024])
            nc.gpsimd.dma_start(out=xv[:, 4:8, 512:1024], in_=xs[:, 4:8, 512:1024])
            nc.sync.dma_start(out=xv[:, 0:4, 1024:T], in_=xs[:, 0:4, 1024:T])
            nc.gpsimd.dma_start(out=xv[:, 4:8, 1024:T], in_=xs[:, 4:8, 1024:T])
            nc.scalar.dma_start(out=wp_sb[:, :], in_=wpT[:, :])
            va_view = va_sb[:, :].rearrange("p (t h e) -> p t h e", t=NQT, h=HL)
            nc.vector.memset(va_view[:, :, :, D:VA], 1.0)
            nc.vector.memset(rc2_sb[:, :], 1.0)

            # ---- merged QKV + attention + projection pipeline ----
            # The PE executes its queue IN ORDER, so emitting all of QKV
            # before attention serializes them.  Instead: a minimal QKV
            # prefix, then the remaining QKV tiles flow through a
            # deadline-ordered work queue sprinkled into the attention
            # i-loops.  PSUM rings (8 banks): s 2x2, y 2x1, aux 2x1.
            with tc.tile_pool(name="s_ps", bufs=2, space="PSUM") as s_pool, \
                 tc.tile_pool(name="y_ps", bufs=2, space="PSUM") as y_pool, \
                 tc.tile_pool(name="aux_ps", bufs=2, space="PSUM") as aux_pool:
                # PE warm-up: the HAM clock gate only reaches 8/8 after
                # ~3.4us of sustained activity and the first real matmul
                # can't start until the input DMA lands; burn the window on
                # throwaway matmuls over the first-loaded mask.
                wps = aux_pool.tile([P, 512], F32, name="warmps", tag="aux")
                for w in range(10):
                    nc.tensor.matmul(
                        wps[:, :], mk_sb[:, :], wsc_sb[:, :],
                        start=True, stop=True,
                    )

                def warm_fill(n):
                    # throwaway matmuls that keep the HAM duty-cycle high
                    # across a known PE bubble (dependency-free, so they
                    # execute exactly when the queue would otherwise stall)
                    w_ps = aux_pool.tile([P, 512], F32, name="wfps", tag="aux")
                    for _ in range(n):
                        nc.tensor.matmul(
                            w_ps[:, :], mk_sb[:, :], wsc_sb[:, :],
                            start=True, stop=True,
                        )

                def qk_tile(w_sb, dst_sb, oc, tch):
                    ps = aux_pool.tile([P, 512], F32, name="qkps", tag="aux")
                    for kc in range(KC):
                        nc.tensor.matmul(
                            ps[:, :],
                            w_sb[:, oc * OCW + kc * P: oc * OCW + kc * P + P],
                            xT_sb[:, kc * T + tch * 512: kc * T + tch * 512 + 512],
                            start=(kc == 0),
                            stop=(kc == KC - 1),
                        )
                    # DVE eviction: ScalarE runs nothing but exp
                    nc.vector.tensor_copy(
                        dst_sb[:, oc * T + tch * 512: oc * T + tch * 512 + 512],
                        ps[:, :],
                    )

                def v_tile(tt):
                    ps = aux_pool.tile([P, 512], F32, name="vps", tag="aux")
                    for kc in range(KC):
                        nc.tensor.matmul(
                            ps[:, 0:OL],
                            xT_sb[:, kc * T + tt * P: kc * T + tt * P + P],
                            wv_sb[:, kc * OL:(kc + 1) * OL],
                            start=(kc == 0),
                            stop=(kc == KC - 1),
                        )
                    nc.vector.tensor_copy(
                        va_view[:, tt, :, 0:D],
                        ps[:, 0:OL].rearrange("p (h d) -> p h d", h=HL),
                    )

                fine_eng = [nc.sync, nc.scalar]
                fine_ctr = [0]

                def proj_tile(tile_idx, fine=False):
                    tt, ocn = divmod(tile_idx, 2)
                    trow = tt * P
                    if fine and fine_ctr[0] % 4 >= 2:
                        # final burst: alternate PSUM pools (aux + the
                        # now-free y ring) so 4 accumulations are in flight
                        # and the PE never waits on an eviction
                        pr_ps = y_pool.tile([P, 512], F32, name="prps2", tag="yps")
                    else:
                        pr_ps = aux_pool.tile([P, 512], F32, name="prps", tag="aux")
                    for cc in range(2):
                        nc.tensor.matmul(
                            pr_ps[:, :],
                            yT_sb[:, cc * T + trow: cc * T + trow + P],
                            wp_sb[:, cc * C + ocn * 512: cc * C + ocn * 512 + 512],
                            start=(cc == 0),
                            stop=(cc == 1),
                        )
                    if not fine:
                        o_sb = outp.tile([P, 512], BF16, name="osb", tag="osb")
                        nc.vector.tensor_copy(o_sb[:, :], pr_ps[:, :])
                        # alternate DMA-issuing engine: descriptors land in
                        # two hardware queues, so the output tiles drain in
                        # parallel instead of serializing
                        if tile_idx % 2 == 0:
                            nc.sync.dma_start(
                                out=out[trow:trow + P, ocn * 512:(ocn + 1) * 512],
                                in_=o_sb[:, :],
                            )
                        else:
                            nc.scalar.dma_start(
                                out=out[trow:trow + P, ocn * 512:(ocn + 1) * 512],
                                in_=o_sb[:, :],
                            )
                    else:
                        # final burst: evict into the contiguous staging
                        # tile (alternating DVE / ScalarE so eviction keeps
                        # pace with the PE); each tile's DMA is issued on
                        # sync right after its eviction so transfers fire
                        # as soon as their semaphore bumps
                        k = fine_ctr[0]
                        fine_ctr[0] += 1
                        dst = stg_sb[:, k * 512:(k + 1) * 512]
                        if k % 2 == 0:
                            nc.vector.tensor_copy(dst, pr_ps[:, :])
                        else:
                            nc.scalar.copy(dst, pr_ps[:, :])
                            b = k // 2
                            eng = fine_eng[b % 2]
                            eng.dma_start(
                                out=out[trow:trow + P, :],
                                in_=stg_sb[:, b * 1024:(b + 1) * 1024],
                            )

                # minimal prefix: exactly what attention chunk (0,0)'s first
                # S matmul needs (v tiles flow through the work queue -- the
                # first Y matmul only runs ~2 exps later)
                qk_tile(wq_sb, qT_sb, 0, 0)
                qk_tile(wk_sb, kT_sb, 0, 0)

                # the rest of QKV, deadline-ordered by the first chunk that
                # consumes each tile; popped one per attention iteration
                def mk_qk(w_sb, dst_sb, oc, tch):
                    return lambda: qk_tile(w_sb, dst_sb, oc, tch)

                def mk_v(tt):
                    return lambda: v_tile(tt)

                work = [mk_v(0), mk_v(1), mk_v(2), mk_v(3),
                        mk_qk(wq_sb, qT_sb, 1, 0), mk_qk(wk_sb, kT_sb, 1, 0)]
                for tch in (1, 2, 3):
                    work += [mk_qk(wq_sb, qT_sb, 0, tch),
                             mk_qk(wk_sb, kT_sb, 0, tch)]
                    work += [mk_v(tt) for tt in range(4 * tch, 4 * tch + 4)]
                    work += [mk_qk(wq_sb, qT_sb, 1, tch),
                             mk_qk(wk_sb, kT_sb, 1, tch)]
                # units that must be emitted before chunk (j4, hp) starts
                req = {(0, 0): 0, (0, 1): 6, (1, 0): 12, (1, 1): 14,
                       (2, 0): 20, (2, 1): 22, (3, 0): 28, (3, 1): 30}
                seq = [(a, b) for a in range(NQC) for b in range(2)]
                emitted = [0]
                # PE broadcast matmuls + DVE normalize-mults deferred from
                # the previous chunk's tail (emitted at i==1 of the next
                # chunk so the in-order PE queue never parks on the
                # reciprocal chain)
                carry = []

                def pop_work():
                    if work:
                        work.pop(0)()
                        emitted[0] += 1

                for j4 in range(NQC):
                    q0 = j4 * 512
                    for hp in range(2):
                        # flush any not-yet-emitted prerequisites
                        while emitted[0] < req[(j4, hp)]:
                            pop_work()
                        nxt = seq.index((j4, hp)) + 1
                        req_next = req[seq[nxt]] if nxt < len(seq) else 30
                        # previous chunk's projection tiles are sprinkled
                        # into the i-loop below: each proj MM is independent
                        # PE work that fills the S->exp->Y handoff bubble.
                        # hp=0's pend tiles need the chunk that JUST ended
                        # (its gpsimd normalize-mult lands ~1.5us in), so
                        # they start at i>=3; hp=1's pend chunk is a full
                        # pass old and can start at i>=1.
                        pend = (
                            [(j4 - 1) * 8 + hp * 4 + k for k in range(4)]
                            if j4 > 0 else []
                        )
                        pend_start = 3 if hp == 0 else 1
                        # two heads interleaved per k-tile: one shared 2-bank
                        # S tile, one wide exp for both heads (the +352cyc
                        # ACTIVATE pipeline fill amortizes over 1024 cols),
                        # two independent y accumulations.  Doubles the
                        # PE-side work available per ACT op.
                        h0, h1 = 2 * hp, 2 * hp + 1
                        ch = hp
                        y0 = y_pool.tile([P, 512], F32, name="yps0", tag="yps")
                        y1 = y_pool.tile([P, 512], F32, name="yps1", tag="yps")
                        nk = 4 * (j4 + 1)

                        def emit_y(c0, p2, i):
                            for half, y_ps, hh in ((0, y0, h0), (1, y1, h1)):
                                nc.tensor.matmul(
                                    y_ps[0:VA, c0:512],
                                    va_sb[:, (i * HL + hh) * VA:(i * HL + hh) * VA + VA],
                                    p2[:, half * 512 + c0: half * 512 + 512],
                                    start=(i == 0),
                                    stop=(i == nk - 1),
                                )

                        prev_y = None
                        for i in range(nk):
                            m0 = max(0, i - 4 * j4)
                            c0 = P * m0
                            s2 = s_pool.tile([P, 1024], F32, name="sps", tag="sps")
                            for half, po in ((0, 0), (1, 64)):
                                nc.tensor.matmul(
                                    s2[:, half * 512 + c0: half * 512 + 512],
                                    kT_sb[po:po + D, ch * T + i * P: ch * T + i * P + P],
                                    qT_sb[po:po + D, ch * T + q0 + c0: ch * T + q0 + 512],
                                    start=True,
                                    stop=True,
                                )
                            p2 = att.tile([P, 1024], BF16, name="pt", tag="pt")
                            if m0 == 0:
                                nc.scalar.activation(
                                    p2[:, 0:1024], s2[:, 0:1024], Exp, scale=0.125
                                )
                            else:
                                # diagonal: the two live spans are disjoint;
                                # one 3D-AP exp covers both (halves the
                                # +352cyc ACTIVATE fills on the diagonal)
                                s2v = s2[:, :].rearrange("p (h c) -> p h c", h=2)
                                p2v = p2[:, :].rearrange("p (h c) -> p h c", h=2)
                                nc.scalar.activation(
                                    p2v[:, :, c0:512], s2v[:, :, c0:512],
                                    Exp, scale=0.125,
                                )
                            if i >= 4 * j4:
                                for half in range(2):
                                    nc.gpsimd.tensor_mul(
                                        p2[:, half * 512 + c0: half * 512 + c0 + P],
                                        p2[:, half * 512 + c0: half * 512 + c0 + P],
                                        mk_sb[:, :],
                                    )
                            # independent PE work between S(i) and Y(i-1):
                            # a QKV tile (paced so each chunk's inputs are
                            # ready one chunk ahead) or a proj tile.  These
                            # MMs execute while exp(i-1)/exp(i) run, so the
                            # in-order PE queue never parks on a Y waiting
                            # for its exp.
                            if carry and i == 1:
                                while carry:
                                    carry.pop(0)()
                            elif emitted[0] < req_next:
                                pop_work()
                            elif pend and i % 2 == 1 and i >= pend_start:
                                proj_tile(pend.pop(0))
                            elif work and i % 2 == 0:
                                pop_work()
                            if prev_y is not None:
                                emit_y(*prev_y)
                            prev_y = (c0, p2, i)
                        emit_y(*prev_y)
                        for t in pend:
                            proj_tile(t)

                        # tail: evict both heads' y rows 0:64 (frees PSUM),
                        # pull the denominator row from PSUM to partition 0,
                        # fast approximate reciprocal (the custom DVE op
                        # needs matching in/out partitions), cast to bf16.
                        # The broadcast is a PE matmul (ones[1,64] stationary
                        # from the mask's first row x rcb[1,512] moving ->
                        # [64,512] PSUM) and the normalize-mult runs on the
                        # DVE reading that PSUM tile directly.  gpsimd runs
                        # ONLY tensor-tensor multiplies (the causal masks):
                        # mixing in PartitionBroadcast or DMA issues forces
                        # a ~7us DSP library reload per switch.
                        last = (j4 == NQC - 1 and hp == 1)
                        # combined two-head normalization: both heads' y
                        # rows evicted into ONE [128,512] tile (h0 on rows
                        # 0:64, h1 on 64:128, matching the yT layout), both
                        # denominator rows into a [2,512] tile -> one 2-lane
                        # reciprocal + bf16 cast, one select-stationary
                        # broadcast matmul (bsel.T @ rcb2 -> [128,512] PSUM)
                        # and one full-width DVE multiply into yT.
                        ypair = misc.tile([P, 512], F32, name="ysb", tag="ysb")
                        if not last:
                            # evictions first (frees the y PSUM ring for the
                            # next chunk ASAP), then the reciprocal chain;
                            # broadcast+mult deferred into the next chunk
                            nc.vector.tensor_copy(ypair[0:D, :], y0[0:D, :])
                            nc.vector.tensor_copy(ypair[D:2 * D, :], y1[0:D, :])
                        else:
                            # final chunk: y evictions on the now-idle
                            # ScalarE, in parallel with the DVE chain
                            nc.scalar.copy(ypair[0:D, :], y0[0:D, :])
                            nc.scalar.copy(ypair[D:2 * D, :], y1[0:D, :])
                        nc.vector.tensor_copy(rc2_sb[0:1, :], y0[D:VA, :])
                        nc.vector.tensor_copy(rc2_sb[32:33, :], y1[D:VA, :])
                        nc.vector.reciprocal_approx_fast(rc2_sb[:, :], rc2_sb[:, :])
                        nc.vector.tensor_copy(rcb2_sb[:, :], rc2_sb[:, :])

                        def mk_norm(ypair=ypair, ch=ch, q0=q0):
                            def emit():
                                bc_ps = aux_pool.tile(
                                    [P, 512], F32, name="bcps", tag="aux"
                                )
                                nc.tensor.matmul(
                                    bc_ps[:, :], bs_sb[:, :], rcb2_sb[:, :],
                                    start=True, stop=True,
                                )
                                nc.vector.tensor_mul(
                                    yT_sb[:, ch * T + q0: ch * T + q0 + 512],
                                    ypair[:, :],
                                    bc_ps[:, :],
                                )
                            return emit

                        if not last:
                            carry.append(mk_norm())
                        else:
                            # the final chunk's normalization gates the last
                            # projection burst and the kernel end: bridge
                            # the ~2us chain with warm fillers (keeps HAM at
                            # full clock), then the staged burst
                            warm_fill(4)
                            mk_norm()()
                            warm_fill(3)
                            for k in range(8):
                                proj_tile(j4 * 8 + k, fine=True)
    nc.compile()
    return nc


def _get_nc():
    global _nc_cache
    if _nc_cache is None:
        _nc_cache = _build_nc()
    return _nc_cache


def _prepare_in_maps(x, W_qkv, W_proj):
    x = np.asarray(x, np.float32)
    W_qkv = np.asarray(W_qkv, np.float32)
    W_proj = np.asarray(W_proj, np.float32)
    # [r, j] = 1 where j >= r (upper triangle incl diag, in S^T [k, q] layout)
    tri = (np.arange(P)[None, :] >= np.arange(P)[:, None]).astype(np.float32)
    tri = tri.astype(_BF16)
    bsel = np.zeros((D, P), np.float32)
    bsel[0, 0:D] = 1.0
    bsel[32, D:2 * D] = 1.0
    bsel = bsel.astype(_BF16)
    in_maps = []
    for c in range(8):
        b, g = c // 4, c % 4
        r0 = OL * g

        def img(a):
            # [R, Y] with R = n*128 rows -> SBUF image [128, n*Y]
            n = a.shape[0] // P
            return np.ascontiguousarray(
                a.reshape(n, P, a.shape[1]).transpose(1, 0, 2).reshape(P, -1)
            ).astype(_BF16)

        def img_oc(a):
            # [1024, 256] -> oc-major SBUF image [128, oc(2) * kc(8) * 128]
            return np.ascontiguousarray(
                a.reshape(KC, P, 2, P).transpose(1, 2, 0, 3).reshape(P, -1)
            ).astype(_BF16)

        in_maps.append({
            "xT": img(x[b].T),
            "wqT": img_oc(W_qkv[r0:r0 + OL, :].T),
            "wkT": img_oc(W_qkv[C + r0:C + r0 + OL, :].T),
            "wvT": img(W_qkv[2 * C + r0:2 * C + r0 + OL, :].T),
            "wpT": img(W_proj[:, r0:r0 + OL].T),
            "mask_tri": tri,
            "bsel": bsel,
        })
    return in_maps


def _combine(results):
    out = np.zeros((B, T, C), np.float32)
    for c in range(8):
        out[c // 4] += results[c]["out"]
    return out


def kernel(x, W_qkv, W_proj):
    nc = _get_nc()
    in_maps = _prepare_in_maps(x, W_qkv, W_proj)
    try:
        res = bass_utils.run_bass_kernel_spmd(nc, in_maps, core_ids=list(range(8)))
    except Exception:
        # rare transient NRT device errors; one retry
        res = bass_utils.run_bass_kernel_spmd(nc, in_maps, core_ids=list(range(8)))
    return _combine(res.results)


def kernel_traced(x, W_qkv, W_proj, trace_cores=None):
    """Like kernel() but returns (out, exec_time_ns) using an NTFF profile."""
    nc = _get_nc()
    in_maps = _prepare_in_maps(x, W_qkv, W_proj)
    res = bass_utils.run_bass_kernel_spmd(
        nc, in_maps, core_ids=list(range(8)), trace=True, trace_cores=trace_cores
    )
    return _combine(res.results), res.exec_time_ns


# revision 34
# speedup vs baseline: 1.0311x; 1.0227x over previous
"""Trainium2 Bass kernel: causal self-attention (B=2, T=2048, C=1024, H=16, Dh=64).

Sharding: 8 cores = 2 (batch) x 4 (head groups of 4 heads).  Each core gets
x[b] plus the W_qkv rows / W_proj columns for its heads, computes the full
attention + a partial output projection for its batch, and the host sums the
4 partials per batch (tensor-parallel unshard).

All matmuls run in bf16 with f32 PSUM accumulation.  x is passed transposed
(xT = x[b].T) so that:
  qT, kT = Wq @ xT, Wk @ xT     (head dim on partitions)  -- no transposes
  v      = xT.T @ WvT           (natural [T, d] layout)
  S^T    = kT_h(tile).T @ qT_h  ([k, q] layout, 128x512 blocks, the two
           heads of a pair row-tiled into array halves)
  exp on ScalarE (logits are bounded, no max pass needed); causal masking by
  computing only the live columns of each block plus one multiplicative
  [128,128] triangle mask on the diagonal subtile (gpsimd); row sums via a
  ones column appended to V (P@[V|1] accumulates y^T and the softmax
  denominators in one PSUM tile).
  out_partial = y^T.T @ WpT   (f32, DMA'd out).

Schedule: the PE executes its queue IN ORDER, so QKV tiles are emitted
through a deadline-ordered work queue sprinkled one-per-iteration into the
attention i-loops (plus previous-chunk projection tiles), never as a bulk
phase.  This keeps the PE continuously busy and the HAM activity monitor at
K=8/8 (full 2.4GHz clock).  Y matmuls are emitted one iteration behind their
exp (software pipelining) so the in-order PE queue never parks on an
unfinished ACTIVATE.

Input DMAs are split across four issuing engines (sync/scalar/vector/
gpsimd) so the ~6MB prefix lands in parallel hardware queues; the QKV weight
images are oc-major so the first 128-output-column slice of Wq/Wk is one
contiguous transfer and the first matmul can start ~4us in.

Softmax normalization (all chunks): denominator row 64 of the y PSUM tile
goes through reciprocal_approx_fast on the DVE (~5x the iterative-divide
RECIPROCAL; 18 good bits is plenty for a softmax denominator), a gpsimd
partition_broadcast over 64 partitions, and a gpsimd multiply into yT.  No
DRAM round-trip: the old den->DRAM->8-lane-reciprocal->DRAM->broadcast path
parked the DVE queue on a ~4us DMA chain, which blocked the y-PSUM
evictions the next chunk's first Y matmul needed.  The final chunk runs the
same chain with the multiply on the DVE and warm-filler matmuls bridging
the ~3us chain latency (keeps HAM at full clock for the last projection
burst); the burst's output DMAs split at 256-column granularity across
three hardware queues.
"""
import sys
import types

import numpy as np
import ml_dtypes

_BF16 = ml_dtypes.bfloat16


def _install_ntff_hook():
    """Provide antenv.axon_hooks so run_bass_kernel_spmd(trace=True) works."""
    if "antenv.axon_hooks" in sys.modules:
        return
    mod = types.ModuleType("antenv.axon_hooks")
    mod._hook = None

    def set_axon_ntff_profile_hook(h):
        mod._hook = h

    def get_axon_ntff_profile_hook():
        return mod._hook

    mod.set_axon_ntff_profile_hook = set_axon_ntff_profile_hook
    mod.get_axon_ntff_profile_hook = get_axon_ntff_profile_hook
    sys.modules["antenv.axon_hooks"] = mod
    try:
        import antenv

        antenv.axon_hooks = mod
    except Exception:
        pass
    try:
        from trn_agent_boot.trn_boot import _ntff_profile_via_ctypes

        mod.set_axon_ntff_profile_hook(
            _ntff_profile_via_ctypes("/opt/axon/libaxon_pjrt.so")
        )
    except Exception:
        pass


_install_ntff_hook()

import concourse.bacc as bacc
import concourse.mybir as mybir
from concourse import bass_utils
from concourse.tile import TileContext

# no network bucket in this container; keep artifacts local
bass_utils.upload_artifacts = lambda tmpdir: tmpdir

BF16 = mybir.dt.bfloat16
F32 = mybir.dt.float32

B, T, C = 2, 2048, 1024
H, D = 16, 64
HL = 4            # heads per core
OL = HL * D       # 256 local qkv output dim
P = 128
KC = C // P       # 8 contraction chunks
NQT = T // P      # 16 q/k 128-tiles
NQC = T // 512    # 4 q 512-chunks
VA = D + 1        # v columns per head incl. ones column (65)
OCW = KC * P      # 1024: per-oc column block in the oc-major weight image

_nc_cache = None


def _build_nc():
    nc = bacc.Bacc("TRN2", target_bir_lowering=False, debug=False, num_devices=8)

    # all inputs arrive pre-arranged in SBUF-image layout [128, X] so every
    # input DMA moves multi-KB contiguous runs per partition row.
    # wq/wk are oc-major ([p, oc, kc, 128]) so the oc=0 half is contiguous.
    xT = nc.declare_dram_parameter("xT", [P, KC * T], BF16, isOutput=False)
    wqT = nc.declare_dram_parameter("wqT", [P, 2 * OCW], BF16, isOutput=False)
    wkT = nc.declare_dram_parameter("wkT", [P, 2 * OCW], BF16, isOutput=False)
    wvT = nc.declare_dram_parameter("wvT", [P, KC * OL], BF16, isOutput=False)
    wpT = nc.declare_dram_parameter("wpT", [P, 2 * C], BF16, isOutput=False)
    mk = nc.declare_dram_parameter("mask_tri", [P, P], BF16, isOutput=False)
    # head-pair select matrix for the broadcast matmul: row 0 = 1 on cols
    # 0:64, row 32 = 1 on cols 64:128, all else 0 (engine APs must start at
    # partition multiples of 32, so the two reciprocal rows live at
    # partitions 0 and 32 of a [64,512] tile)
    bsel = nc.declare_dram_parameter("bsel", [D, P], BF16, isOutput=False)
    # bf16 partials (summed in f32 on the host): halves eviction + output
    # DMA cost; adds ~0.4% relative rounding, well inside the 2e-2 budget
    out = nc.declare_dram_parameter("out", [T, C], BF16, isOutput=True)

    Exp = mybir.ActivationFunctionType.Exp

    with TileContext(nc) as tc:
        with tc.tile_pool(name="const", bufs=1) as const, \
             tc.tile_pool(name="misc", bufs=3) as misc, \
             tc.tile_pool(name="att", bufs=8) as att, \
             tc.tile_pool(name="outp", bufs=6) as outp:
            xT_sb = const.tile([P, KC * T], BF16, name="xT_sb")
            wq_sb = const.tile([P, 2 * OCW], BF16, name="wq_sb")
            wk_sb = const.tile([P, 2 * OCW], BF16, name="wk_sb")
            wv_sb = const.tile([P, KC * OL], BF16, name="wv_sb")
            wp_sb = const.tile([P, 2 * C], BF16, name="wp_sb")
            mk_sb = const.tile([P, P], BF16, name="mk_sb")
            qT_sb = const.tile([P, 2 * T], BF16, name="qT_sb")
            kT_sb = const.tile([P, 2 * T], BF16, name="kT_sb")
            va_sb = const.tile([P, NQT * HL * VA], BF16, name="va_sb")
            yT_sb = const.tile([P, 2 * T], BF16, name="yT_sb")
            bs_sb = const.tile([D, P], BF16, name="bs_sb")
            # contiguous staging for the final projection burst: 4 row
            # blocks x 1024 cols land as 4 clean [128,1024] DMAs
            stg_sb = const.tile([P, 4 * C], BF16, name="stg_sb")
            # persistent reciprocal tiles: rows 0 and 32 carry the two
            # heads' denominators; all other rows stay 1.0 so the shared
            # in-place reciprocal + cast never see garbage
            rc2_sb = const.tile([D, 512], F32, name="rc2_sb")
            rcb2_sb = const.tile([D, 512], BF16, name="rcb2_sb")
            wsc_sb = misc.tile([P, 512], BF16, name="wsc_sb", tag="wsc")

            # ---- input DMAs: issued across THREE engines (sync/scalar/
            # gpsimd; the DVE cannot issue DMAs) so the prefix lands in
            # parallel hardware queues.  First-needed tiles first: mask
            # (warmups), wq oc=0, x tch=0 (split 4 ways), wk oc=0.
            xv = xT_sb[:, :].rearrange("p (n t) -> p n t", n=KC)
            xs = xT[:, :].rearrange("p (n t) -> p n t", n=KC)
            nc.vector.memset(wsc_sb[:, :], 0.0)
            nc.sync.dma_start(out=mk_sb[:, :], in_=mk[:, :])
            nc.scalar.dma_start(out=wq_sb[:, 0:OCW], in_=wqT[:, 0:OCW])
            nc.gpsimd.dma_start(out=wk_sb[:, 0:OCW], in_=wkT[:, 0:OCW])
            nc.sync.dma_start(out=xv[:, 0:3, 0:512], in_=xs[:, 0:3, 0:512])
            nc.scalar.dma_start(out=xv[:, 3:6, 0:512], in_=xs[:, 3:6, 0:512])
            nc.gpsimd.dma_start(out=xv[:, 6:8, 0:512], in_=xs[:, 6:8, 0:512])
            nc.sync.dma_start(out=wq_sb[:, OCW:], in_=wqT[:, OCW:])
            nc.gpsimd.dma_start(out=wv_sb[:, :], in_=wvT[:, :])
            nc.scalar.dma_start(out=wk_sb[:, OCW:], in_=wkT[:, OCW:])
            nc.scalar.dma_start(out=bs_sb[:, :], in_=bsel[:, :])
            nc.sync.dma_start(out=xv[:, 0:4, 512:1024], in_=xs[:, 0:4, 512:1024])
            nc.scalar.dma_start(out=xv[:, 4:8, 512:1024], in_=xs[:, 4:8, 512:1024])
            nc.sync.dma_start(out=xv[:, 0:4, 1024:1536], in_=xs[:, 0:4, 1024:1536])
            nc.gpsimd.dma_start(out=xv[:, 4:8, 1024:1536], in_=xs[:, 4:8, 1024:1536])
            nc.sync.dma_start(out=xv[:, 0:4, 1536:T], in_=xs[:, 0:4, 1536:T])
            nc.gpsimd.dma_start(out=xv[:, 4:8, 1536:T], in_=xs[:, 4:8, 1536:T])
            nc.scalar.dma_start(out=wp_sb[:, :], in_=wpT[:, :])
            va_view = va_sb[:, :].rearrange("p (t h e) -> p t h e", t=NQT, h=HL)
            nc.vector.memset(va_view[:, :, :, D:VA], 1.0)
            nc.vector.memset(rc2_sb[:, :], 1.0)

            # ---- merged QKV + attention + projection pipeline ----
            # The PE executes its queue IN ORDER, so emitting all of QKV
            # before attention serializes them.  Instead: a minimal QKV
            # prefix, then the remaining QKV tiles flow through a
            # deadline-ordered work queue sprinkled into the attention
            # i-loops.  PSUM rings (8 banks): s 2x2, y 2x1, aux 2x1.
            with tc.tile_pool(name="s_ps", bufs=2, space="PSUM") as s_pool, \
                 tc.tile_pool(name="y_ps", bufs=2, space="PSUM") as y_pool, \
                 tc.tile_pool(name="aux_ps", bufs=2, space="PSUM") as aux_pool:
                # PE warm-up: the HAM clock gate only reaches 8/8 after
                # ~3.4us of sustained activity and the first real matmul
                # can't start until the input DMA lands; burn the window on
                # throwaway matmuls over the first-loaded mask.
                wps = aux_pool.tile([P, 512], F32, name="warmps", tag="aux")
                for w in range(8):
                    nc.tensor.matmul(
                        wps[:, :], mk_sb[:, :], wsc_sb[:, :],
                        start=True, stop=True,
                    )

                def warm_fill(n):
                    # throwaway matmuls that keep the HAM duty-cycle high
                    # across a known PE bubble (dependency-free, so they
                    # execute exactly when the queue would otherwise stall)
                    w_ps = aux_pool.tile([P, 512], F32, name="wfps", tag="aux")
                    for _ in range(n):
                        nc.tensor.matmul(
                            w_ps[:, :], mk_sb[:, :], wsc_sb[:, :],
                            start=True, stop=True,
                        )

                def qk_tile(w_sb, dst_sb, oc, tch):
                    ps = aux_pool.tile([P, 512], F32, name="qkps", tag="aux")
                    for kc in range(KC):
                        nc.tensor.matmul(
                            ps[:, :],
                            w_sb[:, oc * OCW + kc * P: oc * OCW + kc * P + P],
                            xT_sb[:, kc * T + tch * 512: kc * T + tch * 512 + 512],
                            start=(kc == 0),
                            stop=(kc == KC - 1),
                        )
                    # DVE eviction: ScalarE runs nothing but exp
                    nc.vector.tensor_copy(
                        dst_sb[:, oc * T + tch * 512: oc * T + tch * 512 + 512],
                        ps[:, :],
                    )

                def v_tile(tt):
                    ps = aux_pool.tile([P, 512], F32, name="vps", tag="aux")
                    for kc in range(KC):
                        nc.tensor.matmul(
                            ps[:, 0:OL],
                            xT_sb[:, kc * T + tt * P: kc * T + tt * P + P],
                            wv_sb[:, kc * OL:(kc + 1) * OL],
                            start=(kc == 0),
                            stop=(kc == KC - 1),
                        )
                    nc.vector.tensor_copy(
                        va_view[:, tt, :, 0:D],
                        ps[:, 0:OL].rearrange("p (h d) -> p h d", h=HL),
                    )

                fine_eng = [nc.sync, nc.scalar]
                fine_ctr = [0]

                def proj_tile(tile_idx, fine=False):
                    tt, ocn = divmod(tile_idx, 2)
                    trow = tt * P
                    if fine and fine_ctr[0] % 4 >= 2:
                        # final burst: alternate PSUM pools (aux + the
                        # now-free y ring) so 4 accumulations are in flight
                        # and the PE never waits on an eviction
                        pr_ps = y_pool.tile([P, 512], F32, name="prps2", tag="yps")
                    else:
                        pr_ps = aux_pool.tile([P, 512], F32, name="prps", tag="aux")
                    for cc in range(2):
                        nc.tensor.matmul(
                            pr_ps[:, :],
                            yT_sb[:, cc * T + trow: cc * T + trow + P],
                            wp_sb[:, cc * C + ocn * 512: cc * C + ocn * 512 + 512],
                            start=(cc == 0),
                            stop=(cc == 1),
                        )
                    if not fine:
                        o_sb = outp.tile([P, 512], BF16, name="osb", tag="osb")
                        nc.vector.tensor_copy(o_sb[:, :], pr_ps[:, :])
                        # alternate DMA-issuing engine: descriptors land in
                        # two hardware queues, so the output tiles drain in
                        # parallel instead of serializing
                        if tile_idx % 2 == 0:
                            nc.sync.dma_start(
                                out=out[trow:trow + P, ocn * 512:(ocn + 1) * 512],
                                in_=o_sb[:, :],
                            )
                        else:
                            nc.scalar.dma_start(
                                out=out[trow:trow + P, ocn * 512:(ocn + 1) * 512],
                                in_=o_sb[:, :],
                            )
                    else:
                        # final burst: evict into the contiguous staging
                        # tile (alternating DVE / ScalarE so eviction keeps
                        # pace with the PE); each tile's DMA is issued on
                        # sync right after its eviction so transfers fire
                        # as soon as their semaphore bumps
                        k = fine_ctr[0]
                        fine_ctr[0] += 1
                        dst = stg_sb[:, k * 512:(k + 1) * 512]
                        if k % 2 == 0:
                            nc.vector.tensor_copy(dst, pr_ps[:, :])
                        else:
                            nc.scalar.copy(dst, pr_ps[:, :])
                            b = k // 2
                            eng = fine_eng[b % 2]
                            eng.dma_start(
                                out=out[trow:trow + P, :],
                                in_=stg_sb[:, b * 1024:(b + 1) * 1024],
                            )

                # minimal prefix: exactly what attention chunk (0,0)'s first
                # S matmul needs (v tiles flow through the work queue -- the
                # first Y matmul only runs ~2 exps later)
                qk_tile(wq_sb, qT_sb, 0, 0)
                qk_tile(wk_sb, kT_sb, 0, 0)

                # the rest of QKV, deadline-ordered by the first chunk that
                # consumes each tile; popped one per attention iteration
                def mk_qk(w_sb, dst_sb, oc, tch):
                    return lambda: qk_tile(w_sb, dst_sb, oc, tch)

                def mk_v(tt):
                    return lambda: v_tile(tt)

                work = [mk_v(0), mk_v(1), mk_v(2), mk_v(3),
                        mk_qk(wq_sb, qT_sb, 1, 0), mk_qk(wk_sb, kT_sb, 1, 0)]
                for tch in (1, 2, 3):
                    work += [mk_qk(wq_sb, qT_sb, 0, tch),
                             mk_qk(wk_sb, kT_sb, 0, tch)]
                    work += [mk_v(tt) for tt in range(4 * tch, 4 * tch + 4)]
                    work += [mk_qk(wq_sb, qT_sb, 1, tch),
                             mk_qk(wk_sb, kT_sb, 1, tch)]
                # units that must be emitted before chunk (j4, hp) starts
                req = {(0, 0): 0, (0, 1): 6, (1, 0): 12, (1, 1): 14,
                       (2, 0): 20, (2, 1): 22, (3, 0): 28, (3, 1): 30}
                seq = [(a, b) for a in range(NQC) for b in range(2)]
                emitted = [0]
                # PE broadcast matmuls + DVE normalize-mults deferred from
                # the previous chunk's tail (emitted at i==1 of the next
                # chunk so the in-order PE queue never parks on the
                # reciprocal chain)
                carry = []

                def pop_work():
                    if work:
                        work.pop(0)()
                        emitted[0] += 1

                def emit_S_exp(j4, hp, i):
                    # one attention iteration's S pair + exp + diag masks;
                    # returns the p2 probability tile.  Factored out so the
                    # next chunk's (i=0) unit can be pre-emitted before the
                    # current chunk's last Y pair (cross-chunk software
                    # pipelining: exp(0') completes during the tail, so the
                    # next chunk's first Y never stalls the in-order PE).
                    q0 = j4 * 512
                    ch = hp
                    m0 = max(0, i - 4 * j4)
                    c0 = P * m0
                    s2 = s_pool.tile([P, 1024], F32, name="sps", tag="sps")
                    for half, po in ((0, 0), (1, 64)):
                        nc.tensor.matmul(
                            s2[:, half * 512 + c0: half * 512 + 512],
                            kT_sb[po:po + D, ch * T + i * P: ch * T + i * P + P],
                            qT_sb[po:po + D, ch * T + q0 + c0: ch * T + q0 + 512],
                            start=True,
                            stop=True,
                        )
                    p2 = att.tile([P, 1024], BF16, name="pt", tag="pt")
                    if m0 == 0:
                        nc.scalar.activation(
                            p2[:, 0:1024], s2[:, 0:1024], Exp, scale=0.125
                        )
                    else:
                        # diagonal: the two live spans are disjoint; one
                        # 3D-AP exp covers both (halves the +352cyc
                        # ACTIVATE fills on the diagonal)
                        s2v = s2[:, :].rearrange("p (h c) -> p h c", h=2)
                        p2v = p2[:, :].rearrange("p (h c) -> p h c", h=2)
                        nc.scalar.activation(
                            p2v[:, :, c0:512], s2v[:, :, c0:512],
                            Exp, scale=0.125,
                        )
                    if i >= 4 * j4:
                        for half in range(2):
                            nc.gpsimd.tensor_mul(
                                p2[:, half * 512 + c0: half * 512 + c0 + P],
                                p2[:, half * 512 + c0: half * 512 + c0 + P],
                                mk_sb[:, :],
                            )
                    return c0, p2

                pre = None  # pre-emitted (c0, p2) for the next chunk's i=0
                for j4 in range(NQC):
                    q0 = j4 * 512
                    for hp in range(2):
                        # flush any not-yet-emitted prerequisites
                        while emitted[0] < req[(j4, hp)]:
                            pop_work()
                        nxt = seq.index((j4, hp)) + 1
                        req_next = req[seq[nxt]] if nxt < len(seq) else 30
                        # previous chunk's projection tiles are sprinkled
                        # into the i-loop below: each proj MM is independent
                        # PE work that fills the S->exp->Y handoff bubble.
                        # hp=0's pend tiles need the chunk that JUST ended
                        # (its gpsimd normalize-mult lands ~1.5us in), so
                        # they start at i>=3; hp=1's pend chunk is a full
                        # pass old and can start at i>=1.
                        pend = (
                            [(j4 - 1) * 8 + hp * 4 + k for k in range(4)]
                            if j4 > 0 else []
                        )
                        pend_start = 3 if hp == 0 else 1
                        # two heads interleaved per k-tile: one shared 2-bank
                        # S tile, one wide exp for both heads (the +352cyc
                        # ACTIVATE pipeline fill amortizes over 1024 cols),
                        # two independent y accumulations.  Doubles the
                        # PE-side work available per ACT op.
                        h0, h1 = 2 * hp, 2 * hp + 1
                        ch = hp
                        y0 = y_pool.tile([P, 512], F32, name="yps0", tag="yps")
                        y1 = y_pool.tile([P, 512], F32, name="yps1", tag="yps")
                        nk = 4 * (j4 + 1)

                        def emit_y(c0, p2, i):
                            for half, y_ps, hh in ((0, y0, h0), (1, y1, h1)):
                                nc.tensor.matmul(
                                    y_ps[0:VA, c0:512],
                                    va_sb[:, (i * HL + hh) * VA:(i * HL + hh) * VA + VA],
                                    p2[:, half * 512 + c0: half * 512 + 512],
                                    start=(i == 0),
                                    stop=(i == nk - 1),
                                )

                        prev_y = None
                        for i in range(nk):
                            if i == 0 and pre is not None:
                                c0, p2 = pre
                                pre = None
                            else:
                                c0, p2 = emit_S_exp(j4, hp, i)
                            # independent PE work between S(i) and Y(i-1):
                            # a QKV tile (paced so each chunk's inputs are
                            # ready one chunk ahead) or a proj tile.  These
                            # MMs execute while exp(i-1)/exp(i) run, so the
                            # in-order PE queue never parks on a Y waiting
                            # for its exp.
                            if carry and i == 1:
                                while carry:
                                    carry.pop(0)()
                            elif emitted[0] < req_next:
                                pop_work()
                            elif pend and i % 2 == 1 and i >= pend_start:
                                proj_tile(pend.pop(0))
                            elif work and i % 2 == 0:
                                pop_work()
                            if i == nk - 1 and nxt < len(seq) \
                                    and emitted[0] >= req[seq[nxt]]:
                                # pre-emit the next chunk's first S pair +
                                # exp between this chunk's last two Y pairs
                                pre = emit_S_exp(*seq[nxt], 0)
                            if prev_y is not None:
                                emit_y(*prev_y)
                            prev_y = (c0, p2, i)
                        emit_y(*prev_y)
                        for t in pend:
                            proj_tile(t)

                        # tail: evict both heads' y rows 0:64 (frees PSUM),
                        # pull the denominator row from PSUM to partition 0,
                        # fast approximate reciprocal (the custom DVE op
                        # needs matching in/out partitions), cast to bf16.
                        # The broadcast is a PE matmul (ones[1,64] stationary
                        # from the mask's first row x rcb[1,512] moving ->
                        # [64,512] PSUM) and the normalize-mult runs on the
                        # DVE reading that PSUM tile directly.  gpsimd runs
                        # ONLY tensor-tensor multiplies (the causal masks):
                        # mixing in PartitionBroadcast or DMA issues forces
                        # a ~7us DSP library reload per switch.
                        last = (j4 == NQC - 1 and hp == 1)
                        # combined two-head normalization: both heads' y
                        # rows evicted into ONE [128,512] tile (h0 on rows
                        # 0:64, h1 on 64:128, matching the yT layout), both
                        # denominator rows into a [2,512] tile -> one 2-lane
                        # reciprocal + bf16 cast, one select-stationary
                        # broadcast matmul (bsel.T @ rcb2 -> [128,512] PSUM)
                        # and one full-width DVE multiply into yT.
                        ypair = misc.tile([P, 512], F32, name="ysb", tag="ysb")
                        if not last:
                            # evictions first (frees the y PSUM ring for the
                            # next chunk ASAP), then the reciprocal chain;
                            # broadcast+mult deferred into the next chunk
                            nc.vector.tensor_copy(ypair[0:D, :], y0[0:D, :])
                            nc.vector.tensor_copy(ypair[D:2 * D, :], y1[0:D, :])
                        else:
                            # final chunk: y evictions on the now-idle
                            # ScalarE, in parallel with the DVE chain
                            nc.scalar.copy(ypair[0:D, :], y0[0:D, :])
                            nc.scalar.copy(ypair[D:2 * D, :], y1[0:D, :])
                        nc.vector.tensor_copy(rc2_sb[0:1, :], y0[D:VA, :])
                        nc.vector.tensor_copy(rc2_sb[32:33, :], y1[D:VA, :])
                        nc.vector.reciprocal_approx_fast(rc2_sb[:, :], rc2_sb[:, :])
                        nc.vector.tensor_copy(rcb2_sb[:, :], rc2_sb[:, :])

                        def mk_norm(ypair=ypair, ch=ch, q0=q0):
                            def emit():
                                bc_ps = aux_pool.tile(
                                    [P, 512], F32, name="bcps", tag="aux"
                                )
                                nc.tensor.matmul(
                                    bc_ps[:, :], bs_sb[:, :], rcb2_sb[:, :],
                                    start=True, stop=True,
                                )
                                nc.vector.tensor_mul(
                                    yT_sb[:, ch * T + q0: ch * T + q0 + 512],
                                    ypair[:, :],
                                    bc_ps[:, :],
                                )
                            return emit

                        if not last:
                            carry.append(mk_norm())
                        else:
                            # the final chunk's normalization gates the last
                            # projection burst and the kernel end: bridge
                            # the ~2us chain with warm fillers (keeps HAM at
                            # full clock), then the staged burst
                            warm_fill(4)
                            mk_norm()()
                            warm_fill(3)
                            for k in range(8):
                                proj_tile(j4 * 8 + k, fine=True)
    nc.compile()
    return nc


def _get_nc():
    global _nc_cache
    if _nc_cache is None:
        _nc_cache = _build_nc()
    return _nc_cache


def _prepare_in_maps(x, W_qkv, W_proj):
    x = np.asarray(x, np.float32)
    W_qkv = np.asarray(W_qkv, np.float32)
    W_proj = np.asarray(W_proj, np.float32)
    # [r, j] = 1 where j >= r (upper triangle incl diag, in S^T [k, q] layout)
    tri = (np.arange(P)[None, :] >= np.arange(P)[:, None]).astype(np.float32)
    tri = tri.astype(_BF16)
    bsel = np.zeros((D, P), np.float32)
    bsel[0, 0:D] = 1.0
    bsel[32, D:2 * D] = 1.0
    bsel = bsel.astype(_BF16)
    in_maps = []
    for c in range(8):
        b, g = c // 4, c % 4
        r0 = OL * g

        def img(a):
            # [R, Y] with R = n*128 rows -> SBUF image [128, n*Y]
            n = a.shape[0] // P
            return np.ascontiguousarray(
                a.reshape(n, P, a.shape[1]).transpose(1, 0, 2).reshape(P, -1)
            ).astype(_BF16)

        def img_oc(a):
            # [1024, 256] -> oc-major SBUF image [128, oc(2) * kc(8) * 128]
            return np.ascontiguousarray(
                a.reshape(KC, P, 2, P).transpose(1, 2, 0, 3).reshape(P, -1)
            ).astype(_BF16)

        in_maps.append({
            "xT": img(x[b].T),
            "wqT": img_oc(W_qkv[r0:r0 + OL, :].T),
            "wkT": img_oc(W_qkv[C + r0:C + r0 + OL, :].T),
            "wvT": img(W_qkv[2 * C + r0:2 * C + r0 + OL, :].T),
            "wpT": img(W_proj[:, r0:r0 + OL].T),
            "mask_tri": tri,
            "bsel": bsel,
        })
    return in_maps


def _combine(results):
    out = np.zeros((B, T, C), np.float32)
    for c in range(8):
        out[c // 4] += results[c]["out"]
    return out


def kernel(x, W_qkv, W_proj):
    nc = _get_nc()
    in_maps = _prepare_in_maps(x, W_qkv, W_proj)
    try:
        res = bass_utils.run_bass_kernel_spmd(nc, in_maps, core_ids=list(range(8)))
    except Exception:
        # rare transient NRT device errors; one retry
        res = bass_utils.run_bass_kernel_spmd(nc, in_maps, core_ids=list(range(8)))
    return _combine(res.results)


def kernel_traced(x, W_qkv, W_proj, trace_cores=None):
    """Like kernel() but returns (out, exec_time_ns) using an NTFF profile."""
    nc = _get_nc()
    in_maps = _prepare_in_maps(x, W_qkv, W_proj)
    res = bass_utils.run_bass_kernel_spmd(
        nc, in_maps, core_ids=list(range(8)), trace=True, trace_cores=trace_cores
    )
    return _combine(res.results), res.exec_time_ns
